# revision 20
# baseline (speedup 1.0000x reference)
"""Trainium2 Bass kernel for nn_Blur: upfirdn2d(up=2, k=4x4 separable binomial).

Math: per (n,c) plane X [128,128] the output is out = A.T @ X @ A with
A [128,255] the 1D polyphase upsampling matrix (2 taps per output row).

Layout insight (from HW benchmarks): output DMA must write large contiguous
per-partition runs, so PLANES live on the partition dim at output time.
Per 128-plane window:
  - H-pass on PE: one fp32 matmul per input column w and y-half:
      psum[g, y] = X[:, :, w].T @ A'[:, yhalf]   (lhsT = X cols, M=planes)
  - ACT drains whole psum banks into S[g, w, y] (SBUF).
  - W-pass on DVE: two fused scalar_tensor_tensor ops per 16-row chunk:
      out[g, y, 2j]   = S[g,j,y] + r*S[g,j+1,y]
      out[g, y, 2j+1] = r*S[g,j,y] + S[g,j+1,y]     (r = v3/v1, v1 folded in A)
    plus x=254 boundary on GPSIMD.
  - Output DMA: [g, 16y, 255x] -> per-partition contiguous ~16KB runs
    (335 GB/s measured vs 41 GB/s for the naive y-on-partition layout).
Sharding: pure data parallel over batch, 2 images (256 planes) per core.
"""

import math

import numpy as np

import concourse.bacc as bacc
import concourse.mybir as mybir
import concourse.tile as tile
from concourse.bass_utils import run_bass_kernel_spmd

N_CORES = 8
N, C, H, W = 16, 128, 128, 128
HO = 2 * H - 1  # 255
PLANES_PER_CORE = (N // N_CORES) * C  # 256
WINDOW = 128  # planes per window (= output DMA partition span)
QLEN = 16  # output rows per staging tile / DMA
DT = mybir.dt.float32


def _taps_from_kernel(kernel2d: np.ndarray) -> np.ndarray:
    """Recover the 1D taps v (kernel2d == outer(v, v))."""
    k = np.asarray(kernel2d, dtype=np.float64)
    assert k.shape == (4, 4)
    v0 = math.sqrt(k[0, 0])
    v = k[0] / v0
    assert np.allclose(np.outer(v, v), k, rtol=1e-6), "kernel is not rank-1"
    assert abs(v[0] - v[3]) < 1e-12 and abs(v[1] - v[2]) < 1e-12, (
        "kernel taps not symmetric"
    )
    return v


def _build_amat(v: np.ndarray) -> np.ndarray:
    """A' = v1 * A, where A [128, 255] maps input rows to upsampled rows."""
    A = np.zeros((H, HO), dtype=np.float64)
    for y in range(HO):
        if y % 2 == 0:
            r = y // 2
            A[r, y] += v[1]
            if r + 1 < H:
                A[r + 1, y] += v[3]
        else:
            A[(y - 1) // 2, y] += v[0]
            A[(y + 1) // 2, y] += v[2]
    return (v[1] * A).astype(np.float32)


def _chunks(total: int, step: int):
    return [(s, min(step, total - s)) for s in range(0, total, step)]


def _build_bass(ratio: float, loop: int = 1, internal_out: bool = False):
    """Trace + compile the per-core Tile program. ratio = v3/v1."""
    nc = bacc.Bacc(
        "TRN2", target_bir_lowering=False, debug=False, num_devices=N_CORES
    )
    amat_d = nc.dram_tensor("amat", [H, HO], DT, kind="ExternalInput")
    if internal_out:
        # timing-only build: no big tensors cross the host link
        imgs_d = nc.dram_tensor("imgs_t", [PLANES_PER_CORE, H, W], DT)
        out_d = nc.dram_tensor("out", [PLANES_PER_CORE, HO, HO], DT)
        done_d = nc.dram_tensor("done", [1, 4], DT, kind="ExternalOutput")
    else:
        imgs_d = nc.dram_tensor(
            "imgs", [PLANES_PER_CORE, H, W], DT, kind="ExternalInput"
        )
        out_d = nc.dram_tensor(
            "out", [PLANES_PER_CORE, HO, HO], DT, kind="ExternalOutput"
        )
        done_d = None

    mult = mybir.AluOpType.mult
    add = mybir.AluOpType.add

    with tile.TileContext(nc) as tc:
        with (
            tc.tile_pool(name="const", bufs=1) as const_pool,
            tc.tile_pool(name="xin", bufs=1) as in_pool,
            tc.tile_pool(name="psum", bufs=8, space="PSUM") as psum_pool,
            tc.tile_pool(name="sblk", bufs=1) as s_pool,
            tc.tile_pool(name="outp", bufs=3) as out_pool,
        ):
            a1 = const_pool.tile([H, 128], DT)
            a2 = const_pool.tile([H, 127], DT)
            nc.sync.dma_start(a1[:], amat_d[:, 0:128])
            nc.sync.dma_start(a2[:], amat_d[:, 128:HO])

            def half_body(g0, x, y0, ylen, ach, win):
                # S stored [g, y, w]: stt APs get 4-8B inner strides
                s = s_pool.tile([128, 128, W], DT, tag="s")
                for wb in range(W // 4):
                    ps = psum_pool.tile([128, 4, 128], DT, tag="ps")
                    for wi in range(4):
                        w = 4 * wb + wi
                        nc.tensor.matmul(
                            ps[:, wi, 0:ylen],
                            x[:, :, w],
                            ach[:, 0:ylen],
                            start=True,
                            stop=True,
                        )
                    nc.scalar.copy(
                        s[:, 0:ylen, 4 * wb : 4 * wb + 4],
                        ps[:, :, 0:ylen].transpose([0, 2, 1]),
                    )

                for qs, qlen in _chunks(ylen, QLEN):
                    o = out_pool.tile([128, QLEN, HO], DT, tag="o")
                    q = slice(qs, qs + qlen)
                    sq0 = s[:, q, 0:127]
                    sq1 = s[:, q, 1:128]
                    # x = 2j   (j=0..126):   S[j] + r*S[j+1]
                    # x = 2j+1 (j=0..126): r*S[j] +   S[j+1]
                    nc.vector.scalar_tensor_tensor(
                        o[:, 0:qlen, 0:253:2],
                        sq1, ratio, sq0, op0=mult, op1=add,
                    )
                    nc.vector.scalar_tensor_tensor(
                        o[:, 0:qlen, 1:254:2],
                        sq0, ratio, sq1, op0=mult, op1=add,
                    )
                    # boundary x = 254: S[127]
                    nc.gpsimd.tensor_copy(o[:, 0:qlen, 254], s[:, q, 127])
                    dst = out_d[g0 : g0 + WINDOW]
                    nc.sync.dma_start(
                        dst[:, y0 + qs : y0 + qs + qlen, :],
                        o[:, 0:qlen, :],
                    )

            def window_body(win):
                g0 = win * WINDOW
                x = in_pool.tile([H, WINDOW, W], DT, tag="x")
                for k in range(WINDOW // 16):
                    src = imgs_d[g0 + 16 * k : g0 + 16 * (k + 1)]
                    nc.sync.dma_start(
                        x[:, 16 * k : 16 * (k + 1), :],
                        src.rearrange("g h w -> h g w"),
                    )
                for (y0, ylen), ach in (((0, 128), a1), ((128, 127), a2)):
                    half_body(g0, x, y0, ylen, ach, win)

            def full_body():
                for win in range(PLANES_PER_CORE // WINDOW):
                    window_body(win)

            if loop == 1:
                full_body()
            else:
                with tc.For_i(0, loop) as _:
                    full_body()

            if done_d is not None:
                nc.sync.dma_start(done_d[:], a1[0:1, 0:4])

    nc.compile()
    return nc


_CACHE: dict = {}


def _get_bass(kernel2d: np.ndarray):
    key = np.asarray(kernel2d, dtype=np.float32).tobytes()
    if key not in _CACHE:
        v = _taps_from_kernel(kernel2d)
        amat = _build_amat(v)
        ratio = float(v[3] / v[1])
        _CACHE[key] = (_build_bass(ratio), amat)
    return _CACHE[key]


def run(imgs: np.ndarray, kernel: np.ndarray, **spmd_kwargs):
    """Run on 8 NeuronCores; returns (full_output, BassKernelResults)."""
    imgs = np.ascontiguousarray(np.asarray(imgs, dtype=np.float32))
    assert imgs.shape == (N, C, H, W)
    nc, amat = _get_bass(kernel)

    per = N // N_CORES
    in_maps = [
        {
            "imgs": imgs[i * per : (i + 1) * per].reshape(
                PLANES_PER_CORE, H, W
            ),
            "amat": amat,
        }
        for i in range(N_CORES)
    ]
    res = run_bass_kernel_spmd(nc, in_maps, list(range(N_CORES)), **spmd_kwargs)
    out = np.concatenate(
        [r["out"].reshape(per, C, HO, HO) for r in res.results], axis=0
    )
    return out, res


def kernel(imgs: np.ndarray, kernel: np.ndarray) -> np.ndarray:
    out, _ = run(imgs, kernel)
    return out


# revision 21
# speedup vs baseline: 1.0797x; 1.0797x over previous
"""Trainium2 Bass kernel for nn_Blur: upfirdn2d(up=2, k=4x4 separable binomial).

Math: per (n,c) plane X [128,128] the output is out = A.T @ X @ A with
A [128,255] the 1D polyphase upsampling matrix (2 taps per output row).

Layout insight (from HW benchmarks): output DMA must write large contiguous
per-partition runs, so PLANES live on the partition dim at output time.
Per 128-plane window:
  - H-pass on PE: one fp32 matmul per input column w and y-half:
      psum[g, y] = X[:, :, w].T @ A'[:, yhalf]   (lhsT = X cols, M=planes)
  - ACT drains whole psum banks into S[g, w, y] (SBUF).
  - W-pass on DVE: two fused scalar_tensor_tensor ops per 16-row chunk:
      out[g, y, 2j]   = S[g,j,y] + r*S[g,j+1,y]
      out[g, y, 2j+1] = r*S[g,j,y] + S[g,j+1,y]     (r = v3/v1, v1 folded in A)
    plus x=254 boundary on GPSIMD.
  - Output DMA: [g, 16y, 255x] -> per-partition contiguous ~16KB runs
    (335 GB/s measured vs 41 GB/s for the naive y-on-partition layout).
Sharding: pure data parallel over batch, 2 images (256 planes) per core.
"""

import math

import numpy as np

import concourse.bacc as bacc
import concourse.mybir as mybir
import concourse.tile as tile
from concourse.bass_utils import run_bass_kernel_spmd

N_CORES = 8
N, C, H, W = 16, 128, 128, 128
HO = 2 * H - 1  # 255
PLANES_PER_CORE = (N // N_CORES) * C  # 256
WINDOW = 128  # planes per window (= output DMA partition span)
QLEN = 32  # output rows per staging tile / DMA
DT = mybir.dt.float32


def _taps_from_kernel(kernel2d: np.ndarray) -> np.ndarray:
    """Recover the 1D taps v (kernel2d == outer(v, v))."""
    k = np.asarray(kernel2d, dtype=np.float64)
    assert k.shape == (4, 4)
    v0 = math.sqrt(k[0, 0])
    v = k[0] / v0
    assert np.allclose(np.outer(v, v), k, rtol=1e-6), "kernel is not rank-1"
    assert abs(v[0] - v[3]) < 1e-12 and abs(v[1] - v[2]) < 1e-12, (
        "kernel taps not symmetric"
    )
    return v


def _build_amat(v: np.ndarray) -> np.ndarray:
    """A' = v1 * A, where A [128, 255] maps input rows to upsampled rows."""
    A = np.zeros((H, HO), dtype=np.float64)
    for y in range(HO):
        if y % 2 == 0:
            r = y // 2
            A[r, y] += v[1]
            if r + 1 < H:
                A[r + 1, y] += v[3]
        else:
            A[(y - 1) // 2, y] += v[0]
            A[(y + 1) // 2, y] += v[2]
    return (v[1] * A).astype(np.float32)


def _chunks(total: int, step: int):
    return [(s, min(step, total - s)) for s in range(0, total, step)]


def _build_bass(ratio: float, loop: int = 1, internal_out: bool = False):
    """Trace + compile the per-core Tile program. ratio = v3/v1."""
    nc = bacc.Bacc(
        "TRN2", target_bir_lowering=False, debug=False, num_devices=N_CORES
    )
    amat_d = nc.dram_tensor("amat", [H, HO], DT, kind="ExternalInput")
    if internal_out:
        # timing-only build: no big tensors cross the host link
        imgs_d = nc.dram_tensor("imgs_t", [PLANES_PER_CORE, H, W], DT)
        out_d = nc.dram_tensor("out", [PLANES_PER_CORE, HO, HO], DT)
        done_d = nc.dram_tensor("done", [1, 4], DT, kind="ExternalOutput")
    else:
        imgs_d = nc.dram_tensor(
            "imgs", [PLANES_PER_CORE, H, W], DT, kind="ExternalInput"
        )
        out_d = nc.dram_tensor(
            "out", [PLANES_PER_CORE, HO, HO], DT, kind="ExternalOutput"
        )
        done_d = None

    mult = mybir.AluOpType.mult
    add = mybir.AluOpType.add

    with tile.TileContext(nc) as tc:
        with (
            tc.tile_pool(name="const", bufs=1) as const_pool,
            tc.tile_pool(name="xin", bufs=1) as in_pool,
            tc.tile_pool(name="psum", bufs=8, space="PSUM") as psum_pool,
            tc.tile_pool(name="sblk", bufs=1) as s_pool,
            tc.tile_pool(name="outp", bufs=2) as out_pool,
        ):
            a1 = const_pool.tile([H, 128], DT)
            a2 = const_pool.tile([H, 127], DT)
            nc.sync.dma_start(a1[:], amat_d[:, 0:128])
            nc.sync.dma_start(a2[:], amat_d[:, 128:HO])

            def half_body(g0, x, y0, ylen, ach, win):
                # S stored [g, y, w]: stt APs get 4-8B inner strides
                s = s_pool.tile([128, 128, W], DT, tag="s")
                for wb in range(W // 4):
                    ps = psum_pool.tile([128, 4, 128], DT, tag="ps")
                    for wi in range(4):
                        w = 4 * wb + wi
                        nc.tensor.matmul(
                            ps[:, wi, 0:ylen],
                            x[:, :, w],
                            ach[:, 0:ylen],
                            start=True,
                            stop=True,
                        )
                    nc.scalar.copy(
                        s[:, 0:ylen, 4 * wb : 4 * wb + 4],
                        ps[:, :, 0:ylen].transpose([0, 2, 1]),
                    )

                for qs, qlen in _chunks(ylen, QLEN):
                    o = out_pool.tile([128, QLEN, HO], DT, tag="o")
                    q = slice(qs, qs + qlen)
                    sq0 = s[:, q, 0:127]
                    sq1 = s[:, q, 1:128]
                    # x = 2j   (j=0..126):   S[j] + r*S[j+1]
                    # x = 2j+1 (j=0..126): r*S[j] +   S[j+1]
                    nc.vector.scalar_tensor_tensor(
                        o[:, 0:qlen, 0:253:2],
                        sq1, ratio, sq0, op0=mult, op1=add,
                    )
                    nc.vector.scalar_tensor_tensor(
                        o[:, 0:qlen, 1:254:2],
                        sq0, ratio, sq1, op0=mult, op1=add,
                    )
                    # boundary x = 254: S[127]
                    nc.gpsimd.tensor_copy(o[:, 0:qlen, 254], s[:, q, 127])
                    dst = out_d[g0 : g0 + WINDOW]
                    nc.sync.dma_start(
                        dst[:, y0 + qs : y0 + qs + qlen, :],
                        o[:, 0:qlen, :],
                    )

            def window_body(win):
                g0 = win * WINDOW
                x = in_pool.tile([H, WINDOW, W], DT, tag="x")
                for k in range(WINDOW // 16):
                    src = imgs_d[g0 + 16 * k : g0 + 16 * (k + 1)]
                    nc.sync.dma_start(
                        x[:, 16 * k : 16 * (k + 1), :],
                        src.rearrange("g h w -> h g w"),
                    )
                for (y0, ylen), ach in (((0, 128), a1), ((128, 127), a2)):
                    half_body(g0, x, y0, ylen, ach, win)

            def full_body():
                for win in range(PLANES_PER_CORE // WINDOW):
                    window_body(win)

            if loop == 1:
                full_body()
            else:
                with tc.For_i(0, loop) as _:
                    full_body()

            if done_d is not None:
                nc.sync.dma_start(done_d[:], a1[0:1, 0:4])

    nc.compile()
    return nc


_CACHE: dict = {}


def _get_bass(kernel2d: np.ndarray):
    key = np.asarray(kernel2d, dtype=np.float32).tobytes()
    if key not in _CACHE:
        v = _taps_from_kernel(kernel2d)
        amat = _build_amat(v)
        ratio = float(v[3] / v[1])
        _CACHE[key] = (_build_bass(ratio), amat)
    return _CACHE[key]


def run(imgs: np.ndarray, kernel: np.ndarray, **spmd_kwargs):
    """Run on 8 NeuronCores; returns (full_output, BassKernelResults)."""
    imgs = np.ascontiguousarray(np.asarray(imgs, dtype=np.float32))
    assert imgs.shape == (N, C, H, W)
    nc, amat = _get_bass(kernel)

    per = N // N_CORES
    in_maps = [
        {
            "imgs": imgs[i * per : (i + 1) * per].reshape(
                PLANES_PER_CORE, H, W
            ),
            "amat": amat,
        }
        for i in range(N_CORES)
    ]
    res = run_bass_kernel_spmd(nc, in_maps, list(range(N_CORES)), **spmd_kwargs)
    out = np.concatenate(
        [r["out"].reshape(per, C, HO, HO) for r in res.results], axis=0
    )
    return out, res


def kernel(imgs: np.ndarray, kernel: np.ndarray) -> np.ndarray:
    out, _ = run(imgs, kernel)
    return out


# revision 22
# speedup vs baseline: 1.1305x; 1.0470x over previous
"""Trainium2 Bass kernel for nn_Blur: upfirdn2d(up=2, k=4x4 separable binomial).

Math: per (n,c) plane X [128,128] the output is out = A.T @ X @ A with
A [128,255] the 1D polyphase upsampling matrix (2 taps per output row).

Layout insight (from HW benchmarks): output DMA must write large contiguous
per-partition runs, so PLANES live on the partition dim at output time.
Per 128-plane window:
  - H-pass on PE: one fp32 matmul per input column w and y-half:
      psum[g, y] = X[:, :, w].T @ A'[:, yhalf]   (lhsT = X cols, M=planes)
  - ACT drains whole psum banks into S[g, w, y] (SBUF).
  - W-pass on DVE: two fused scalar_tensor_tensor ops per 16-row chunk:
      out[g, y, 2j]   = S[g,j,y] + r*S[g,j+1,y]
      out[g, y, 2j+1] = r*S[g,j,y] + S[g,j+1,y]     (r = v3/v1, v1 folded in A)
    plus x=254 boundary on GPSIMD.
  - Output DMA: [g, 16y, 255x] -> per-partition contiguous ~16KB runs
    (335 GB/s measured vs 41 GB/s for the naive y-on-partition layout).
Sharding: pure data parallel over batch, 2 images (256 planes) per core.
"""

import math

import numpy as np

import concourse.bacc as bacc
import concourse.mybir as mybir
import concourse.tile as tile
from concourse.bass_utils import run_bass_kernel_spmd

N_CORES = 8
N, C, H, W = 16, 128, 128, 128
HO = 2 * H - 1  # 255
PLANES_PER_CORE = (N // N_CORES) * C  # 256
WINDOW = 128  # planes per window (= output DMA partition span)
QLEN = 16  # output rows per staging tile / DMA
DT = mybir.dt.float32


def _taps_from_kernel(kernel2d: np.ndarray) -> np.ndarray:
    """Recover the 1D taps v (kernel2d == outer(v, v))."""
    k = np.asarray(kernel2d, dtype=np.float64)
    assert k.shape == (4, 4)
    v0 = math.sqrt(k[0, 0])
    v = k[0] / v0
    assert np.allclose(np.outer(v, v), k, rtol=1e-6), "kernel is not rank-1"
    assert abs(v[0] - v[3]) < 1e-12 and abs(v[1] - v[2]) < 1e-12, (
        "kernel taps not symmetric"
    )
    return v


def _build_amat(v: np.ndarray) -> np.ndarray:
    """A' = v1 * A, where A [128, 255] maps input rows to upsampled rows."""
    A = np.zeros((H, HO), dtype=np.float64)
    for y in range(HO):
        if y % 2 == 0:
            r = y // 2
            A[r, y] += v[1]
            if r + 1 < H:
                A[r + 1, y] += v[3]
        else:
            A[(y - 1) // 2, y] += v[0]
            A[(y + 1) // 2, y] += v[2]
    return (v[1] * A).astype(np.float32)


def _chunks(total: int, step: int):
    return [(s, min(step, total - s)) for s in range(0, total, step)]


def _build_bass(ratio: float, loop: int = 1, internal_out: bool = False):
    """Trace + compile the per-core Tile program. ratio = v3/v1."""
    nc = bacc.Bacc(
        "TRN2", target_bir_lowering=False, debug=False, num_devices=N_CORES
    )
    amat_d = nc.dram_tensor("amat", [H, HO], DT, kind="ExternalInput")
    if internal_out:
        # timing-only build: no big tensors cross the host link
        imgs_d = nc.dram_tensor("imgs_t", [PLANES_PER_CORE, H, W], DT)
        out_d = nc.dram_tensor("out", [PLANES_PER_CORE, HO, HO], DT)
        done_d = nc.dram_tensor("done", [1, 4], DT, kind="ExternalOutput")
    else:
        imgs_d = nc.dram_tensor(
            "imgs", [PLANES_PER_CORE, H, W], DT, kind="ExternalInput"
        )
        out_d = nc.dram_tensor(
            "out", [PLANES_PER_CORE, HO, HO], DT, kind="ExternalOutput"
        )
        done_d = None

    mult = mybir.AluOpType.mult
    add = mybir.AluOpType.add

    with tile.TileContext(nc) as tc:
        with (
            tc.tile_pool(name="const", bufs=1) as const_pool,
            tc.tile_pool(name="xin", bufs=1) as in_pool,
            tc.tile_pool(name="psum", bufs=8, space="PSUM") as psum_pool,
            tc.tile_pool(name="sblk", bufs=1) as s_pool,
            tc.tile_pool(name="outp", bufs=3) as out_pool,
        ):
            a1 = const_pool.tile([H, 128], DT)
            a2 = const_pool.tile([H, 127], DT)
            nc.sync.dma_start(a1[:], amat_d[:, 0:128])
            nc.sync.dma_start(a2[:], amat_d[:, 128:HO])

            def half_body(g0, x, y0, ylen, ach, win):
                # S stored [g, y, w]: stt APs get 4-8B inner strides
                s = s_pool.tile([128, 128, W], DT, tag="s")
                for wb in range(W // 4):
                    ps = psum_pool.tile([128, 4, 128], DT, tag="ps")
                    for wi in range(4):
                        w = 4 * wb + wi
                        nc.tensor.matmul(
                            ps[:, wi, 0:ylen],
                            x[:, :, w],
                            ach[:, 0:ylen],
                            start=True,
                            stop=True,
                        )
                    nc.scalar.copy(
                        s[:, 0:ylen, 4 * wb : 4 * wb + 4],
                        ps[:, :, 0:ylen].transpose([0, 2, 1]),
                    )

                for qs, qlen in _chunks(ylen, QLEN):
                    o = out_pool.tile([128, QLEN, HO], DT, tag="o")
                    q = slice(qs, qs + qlen)
                    sq0 = s[:, q, 0:127]
                    sq1 = s[:, q, 1:128]
                    # x = 2j   (j=0..126):   S[j] + r*S[j+1]
                    # x = 2j+1 (j=0..126): r*S[j] +   S[j+1]
                    nc.vector.scalar_tensor_tensor(
                        o[:, 0:qlen, 0:253:2],
                        sq1, ratio, sq0, op0=mult, op1=add,
                    )
                    nc.vector.scalar_tensor_tensor(
                        o[:, 0:qlen, 1:254:2],
                        sq0, ratio, sq1, op0=mult, op1=add,
                    )
                    # boundary x = 254: S[127]
                    nc.gpsimd.tensor_copy(o[:, 0:qlen, 254], s[:, q, 127])
                    dst = out_d[g0 : g0 + WINDOW]
                    nc.sync.dma_start(
                        dst[:, y0 + qs : y0 + qs + qlen, :],
                        o[:, 0:qlen, :],
                    )

            def window_body(win):
                g0 = win * WINDOW
                x = in_pool.tile([H, WINDOW, W], DT, tag="x")
                for k in range(WINDOW // 16):
                    src = imgs_d[g0 + 16 * k : g0 + 16 * (k + 1)]
                    # SWDGE path: separate queue from the SP-issued out DMAs
                    nc.gpsimd.dma_start(
                        x[:, 16 * k : 16 * (k + 1), :],
                        src.rearrange("g h w -> h g w"),
                    )
                for (y0, ylen), ach in (((0, 128), a1), ((128, 127), a2)):
                    half_body(g0, x, y0, ylen, ach, win)

            def full_body():
                for win in range(PLANES_PER_CORE // WINDOW):
                    window_body(win)

            if loop == 1:
                full_body()
            else:
                with tc.For_i(0, loop) as _:
                    full_body()

            if done_d is not None:
                nc.sync.dma_start(done_d[:], a1[0:1, 0:4])

    nc.compile()
    return nc


_CACHE: dict = {}


def _get_bass(kernel2d: np.ndarray):
    key = np.asarray(kernel2d, dtype=np.float32).tobytes()
    if key not in _CACHE:
        v = _taps_from_kernel(kernel2d)
        amat = _build_amat(v)
        ratio = float(v[3] / v[1])
        _CACHE[key] = (_build_bass(ratio), amat)
    return _CACHE[key]


def run(imgs: np.ndarray, kernel: np.ndarray, **spmd_kwargs):
    """Run on 8 NeuronCores; returns (full_output, BassKernelResults)."""
    imgs = np.ascontiguousarray(np.asarray(imgs, dtype=np.float32))
    assert imgs.shape == (N, C, H, W)
    nc, amat = _get_bass(kernel)

    per = N // N_CORES
    in_maps = [
        {
            "imgs": imgs[i * per : (i + 1) * per].reshape(
                PLANES_PER_CORE, H, W
            ),
            "amat": amat,
        }
        for i in range(N_CORES)
    ]
    res = run_bass_kernel_spmd(nc, in_maps, list(range(N_CORES)), **spmd_kwargs)
    out = np.concatenate(
        [r["out"].reshape(per, C, HO, HO) for r in res.results], axis=0
    )
    return out, res


def kernel(imgs: np.ndarray, kernel: np.ndarray) -> np.ndarray:
    out, _ = run(imgs, kernel)
    return out
